# revision 1
# baseline (speedup 1.0000x reference)
"""GQA attention (B=2,S=2048,DIM=2048,H=32,KVH=8,HD=64) + RoPE, causal.

Distributed over 8 TRN2 NeuronCores: core = 4*batch + head_group.
Each core computes attention for its 8 q-heads (2 kv-heads) of one batch,
then chunked AllGathers (overlapped with attention) reshard head-major ->
sequence-major so each core runs the output projection for 512 sequence
rows against the full (permuted) wo. Host-side work is layout-only:
weight column/row permutations, batch split, and concatenation of the
per-core output row-slices.
"""
import numpy as np

import concourse.bass as bass
import concourse.bacc as bacc
import concourse.tile as tile
from concourse.tile import add_dep_helper
import concourse.mybir as mybir
from concourse import bass_utils


def _ensure_axon_hooks_shim():
    """bass_utils imports antenv.axon_hooks when BASS_TRACE is set; the
    module is absent in some images. Provide a no-op shim so tracing env
    vars cannot crash the run."""
    import sys, types
    try:
        import antenv  # noqa
        if "antenv.axon_hooks" in sys.modules:
            return
        import importlib
        try:
            importlib.import_module("antenv.axon_hooks")
            return
        except ImportError:
            pass
        mod = types.ModuleType("antenv.axon_hooks")
        mod._hook = None
        mod.get_axon_ntff_profile_hook = lambda: mod._hook

        def set_axon_ntff_profile_hook(h):
            mod._hook = h
        mod.set_axon_ntff_profile_hook = set_axon_ntff_profile_hook
        sys.modules["antenv.axon_hooks"] = mod
        antenv.axon_hooks = mod
    except Exception:
        pass


_ensure_axon_hooks_shim()

F32 = mybir.dt.float32
BF16 = mybir.dt.bfloat16

B, S, DIM = 2, 2048, 2048
H, KVH, HD = 32, 8, 64
N_CORES = 8
GROUPS = [[0, 1, 2, 3], [4, 5, 6, 7]]
NCH = 4            # sequence chunks (queries) of 512
CHUNK = S // NCH   # 512
SEQT = S // 128    # 16 seq tiles
DT = DIM // 128    # 16 contraction tiles
# q-head slot order inside a core: slot s holds local q-head s//2 + 4*(s%2),
# so slot parity == local kv-head index (kv = local_head // 4).
SLOT_TO_LOCAL = [s // 2 + 4 * (s % 2) for s in range(8)]
# rope pair permutation within one head: evens then odds
HD_PERM = np.concatenate([np.arange(0, HD, 2), np.arange(1, HD, 2)])
PIPELINE_SCALE = True
ENGMOVE = True
TRMOVE = False


def _build():
    nc = bacc.Bacc("TRN2", target_bir_lowering=False, debug=False,
                   num_devices=N_CORES)
    x_d = nc.dram_tensor("x", [S, DIM], F32, kind="ExternalInput")
    wq_d = nc.dram_tensor("wq", [DIM, 512], F32, kind="ExternalInput")
    wkv_d = nc.dram_tensor("wkv", [DIM, 256], F32, kind="ExternalInput")
    wo_d = nc.dram_tensor("wo", [DIM, DIM], F32, kind="ExternalInput")
    cos_d = nc.dram_tensor("cos", [S, HD // 2], F32, kind="ExternalInput")
    sin_d = nc.dram_tensor("sin", [S, HD // 2], F32, kind="ExternalInput")
    goffr_d = nc.dram_tensor("goffr", [1, 1], mybir.dt.uint32, kind="ExternalInput")
    goffc_d = nc.dram_tensor("goffc", [1, 1], mybir.dt.uint32, kind="ExternalInput")
    out_d = nc.dram_tensor("out", [CHUNK, DIM], F32, kind="ExternalOutput")

    Exp = mybir.ActivationFunctionType.Exp
    Copy = mybir.ActivationFunctionType.Copy

    with tile.TileContext(nc) as tc:
        with tc.tile_pool(name="dram", bufs=1, space="DRAM") as dram, \
             tc.tile_pool(name="wpool", bufs=1) as wpool:
            # ---- DRAM scratch ----
            xbf = dram.tile([S, DIM], BF16)
            qnat = dram.tile([S, 512], BF16)
            knat = dram.tile([S, 128], BF16)
            ag_in = dram.tile([2, CHUNK, 2 * CHUNK], BF16)  # halves: heads x 1024 q
            gath = dram.tile([2, S, 2 * CHUNK], BF16)

            # ---- persistent SBUF ----
            wq_sb = wpool.tile([128, DT, 512], BF16)
            wkv_sb = wpool.tile([128, DT, 256], BF16)
            wo_sb = wpool.tile([128, DT, DIM], BF16)
            cos_sb = wpool.tile([128, SEQT, 32], F32)
            sin_sb = wpool.tile([128, SEQT, 32], F32)
            qt_sb = wpool.tile([128, 4, S], BF16)     # Q^T (slot pairs)
            kt_sb = wpool.tile([128, S], BF16)        # K^T (kv0|kv1)
            v_sb = wpool.tile([128, SEQT, 130], BF16)  # [V0|1|V1|1] per key tile
            e_sb = wpool.tile([8, 512], BF16)         # recip expand indicator

            # cos/sin loads
            nc.scalar.dma_start(cos_sb[:], cos_d[:].rearrange("(t p) c -> p t c", p=128))
            nc.scalar.dma_start(sin_sb[:], sin_d[:].rearrange("(t p) c -> p t c", p=128))
            # ones columns of V_aug; indicator matrix
            nc.gpsimd.memset(v_sb[:, :, 64:65], 1.0)
            nc.gpsimd.memset(v_sb[:, :, 129:130], 1.0)
            nc.gpsimd.memset(e_sb[:], 1.0)
            nc.gpsimd.affine_select(
                out=e_sb[:].rearrange("p (s j) -> p s j", s=8),
                in_=e_sb[:].rearrange("p (s j) -> p s j", s=8),
                compare_op=mybir.AluOpType.is_equal,
                fill=0.0, base=0,
                pattern=[[-1, 8], [0, 64]], channel_multiplier=1,
            )

            # stage x chunk 0 first so its transposes beat the weight loads
            with tc.tile_pool(name="x0", bufs=4) as x0p:
                for tt in range(4):
                    xf0 = x0p.tile([128, DIM], F32, tag="xf0", name="xf0")
                    nc.scalar.dma_start(xf0[:], x_d[tt * 128:(tt + 1) * 128, :])
                    xb0 = x0p.tile([128, DIM], BF16, tag="xb0", name="xb0")
                    nc.vector.tensor_copy(xb0[:], xf0[:])
                    nc.sync.dma_start(xbf[tt * 128:(tt + 1) * 128, :], xb0[:])

            # ---- load + cast wq/wkv (cast on ACT) ----
            with tc.tile_pool(name="wtmp", bufs=3) as wtmp:
                for kt in range(DT):
                    wt = wtmp.tile([128, 512 + 256], F32, tag="wf")
                    nc.scalar.dma_start(wt[:, 0:512], wq_d[kt * 128:(kt + 1) * 128, :])
                    nc.scalar.dma_start(wt[:, 512:768], wkv_d[kt * 128:(kt + 1) * 128, :])
                    eng = nc.vector if ENGMOVE else nc.gpsimd
                    eng.tensor_copy(wq_sb[:, kt, :], wt[:, 0:512])
                    eng.tensor_copy(wkv_sb[:, kt, :], wt[:, 512:768])

            # preload the exp table set so it doesn't stall the first QK
            warm = wpool.tile([128, 1], F32)
            nc.gpsimd.memset(warm[:], 0.0)
            nc.scalar.activation(warm[:], warm[:], Exp)

            # ---- x pipeline + projections + rope, per chunk ----
            with tc.tile_pool(name="xio", bufs=2) as xio, \
                 tc.tile_pool(name="ppsum", bufs=2, space="PSUM") as ppsum:
                def qk_transpose(c):
                    for sp in range(4):
                        nc.sync.dma_start_transpose(
                            qt_sb[:, sp, c * CHUNK:(c + 1) * CHUNK],
                            qnat[c * CHUNK:(c + 1) * CHUNK, sp * 128:(sp + 1) * 128])
                    nc.sync.dma_start_transpose(
                        kt_sb[:, c * CHUNK:(c + 1) * CHUNK],
                        knat[c * CHUNK:(c + 1) * CHUNK, :])

                for c in range(NCH):
                    # stage x chunk: f32 -> bf16 (DVE) -> DRAM (for xbar transpose)
                    for tt in range(4 if c > 0 else 0):
                        gt = 4 * c + tt
                        xf = xio.tile([128, DIM], F32, tag="xf", bufs=3)
                        nc.scalar.dma_start(xf[:], x_d[gt * 128:(gt + 1) * 128, :])
                        xb = xio.tile([128, DIM], BF16, tag="xb", bufs=2)
                        (nc.vector if ENGMOVE else nc.gpsimd).tensor_copy(xb[:], xf[:])
                        nc.sync.dma_start(xbf[gt * 128:(gt + 1) * 128, :], xb[:])
                    # transpose chunk of x into x^T tiles (spread over 2 queues)
                    xT = xio.tile([128, DT, CHUNK], BF16, tag="xT", bufs=3)
                    for dt in range(DT):
                        nc.sync.dma_start_transpose(
                            xT[:, dt, :],
                            xbf[c * CHUNK:(c + 1) * CHUNK, dt * 128:(dt + 1) * 128])
                    if c > 0:
                        qk_transpose(c - 1)
                    # projections + rope per seq tile
                    for tt in range(4):
                        gt = 4 * c + tt
                        q_ps = ppsum.tile([128, 512], F32, tag="qps", bufs=3)
                        kv_ps = ppsum.tile([128, 256], F32, tag="kvps", bufs=3)
                        for dt in range(DT):
                            st = xT[:, dt, tt * 128:(tt + 1) * 128]
                            nc.tensor.matmul(q_ps[:], st, wq_sb[:, dt, :],
                                             start=(dt == 0), stop=(dt == DT - 1))
                            nc.tensor.matmul(kv_ps[:], st, wkv_sb[:, dt, :],
                                             start=(dt == 0), stop=(dt == DT - 1))
                        # rope Q: per head [a(32)|b(32)]
                        q4 = q_ps[:].rearrange("p (s two c) -> p s two c", s=8, two=2)
                        cb2 = cos_sb[:, gt, None, None, :].to_broadcast((128, 8, 2, 32))
                        sb = sin_sb[:, gt, None, :].to_broadcast((128, 8, 32))
                        t1 = xio.tile([128, 512], BF16, tag="t1")
                        t2 = xio.tile([128, 512], BF16, tag="t2")
                        qn = xio.tile([128, 512], BF16, tag="qn")
                        t1_4 = t1[:].rearrange("p (s two c) -> p s two c", s=8, two=2)
                        t2_4 = t2[:].rearrange("p (s two c) -> p s two c", s=8, two=2)
                        qn4 = qn[:].rearrange("p (s two c) -> p s two c", s=8, two=2)
                        nc.vector.tensor_mul(t1_4, q4, cb2)
                        nc.vector.tensor_mul(t2_4[:, :, 0, :], q4[:, :, 1, :], sb)
                        nc.vector.tensor_mul(t2_4[:, :, 1, :], q4[:, :, 0, :], sb)
                        nc.vector.tensor_sub(qn4[:, :, 0, :], t1_4[:, :, 0, :], t2_4[:, :, 0, :])
                        nc.vector.tensor_add(qn4[:, :, 1, :], t1_4[:, :, 1, :], t2_4[:, :, 1, :])
                        nc.gpsimd.dma_start(qnat[gt * 128:(gt + 1) * 128, :], qn[:])
                        # rope K (2 kv heads)
                        k4 = kv_ps[:, 0:128].rearrange("p (s two c) -> p s two c", s=2, two=2)
                        cb2k = cos_sb[:, gt, None, None, :].to_broadcast((128, 2, 2, 32))
                        sbk = sin_sb[:, gt, None, :].to_broadcast((128, 2, 32))
                        k1 = xio.tile([128, 128], BF16, tag="k1")
                        k2 = xio.tile([128, 128], BF16, tag="k2")
                        kn = xio.tile([128, 128], BF16, tag="kn")
                        k1_4 = k1[:].rearrange("p (s two c) -> p s two c", s=2, two=2)
                        k2_4 = k2[:].rearrange("p (s two c) -> p s two c", s=2, two=2)
                        kn4 = kn[:].rearrange("p (s two c) -> p s two c", s=2, two=2)
                        nc.vector.tensor_mul(k1_4, k4, cb2k)
                        nc.vector.tensor_mul(k2_4[:, :, 0, :], k4[:, :, 1, :], sbk)
                        nc.vector.tensor_mul(k2_4[:, :, 1, :], k4[:, :, 0, :], sbk)
                        nc.vector.tensor_sub(kn4[:, :, 0, :], k1_4[:, :, 0, :], k2_4[:, :, 0, :])
                        nc.vector.tensor_add(kn4[:, :, 1, :], k1_4[:, :, 1, :], k2_4[:, :, 1, :])
                        nc.gpsimd.dma_start(knat[gt * 128:(gt + 1) * 128, :], kn[:])
                        # V evacuation (no rope)
                        nc.vector.tensor_copy(v_sb[:, gt, 0:64], kv_ps[:, 128:192])
                        nc.vector.tensor_copy(v_sb[:, gt, 65:129], kv_ps[:, 192:256])
                    # transpose roped q/k of this chunk


                qk_transpose(NCH - 1)

            # ---- attention, scaling pipelined one chunk behind ----
            with tc.tile_pool(name="apsum", bufs=1, space="PSUM") as apsum, \
                 tc.tile_pool(name="asb", bufs=2) as asb, \
                 tc.tile_pool(name="wtmp2", bufs=2) as wtmp2:

                def load_wo(kt):
                    wof = wtmp2.tile([128, DIM], F32, tag="wof", name="wof")
                    nc.scalar.dma_start(wof[:], wo_d[kt * 128:(kt + 1) * 128, :])
                    nc.vector.tensor_copy(wo_sb[:, kt, :], wof[:])

                def emit_ag(h):
                    cc = nc.gpsimd.collective_compute(
                        "AllGather", mybir.AluOpType.bypass,
                        replica_groups=GROUPS,
                        ins=[ag_in[h][:, :].opt()], outs=[gath[h][:, :].opt()])
                    for d in ag_dmas[h]:
                        add_dep_helper(cc.ins, d.ins, sync=True,
                                       reason="AG waits its staging DMAs")
                    cc_insts.append(cc)

                def emit_scale(pc, pstages, precipb):
                    """rexp matmuls + scale + ag staging DMA."""
                    for s in range(8):
                        rexp = apsum.tile([128, 512], F32, tag="attn", bufs=2,
                                          name="rexp")
                        nc.tensor.matmul(rexp[0:64, :], e_sb[:, 64 * s:64 * s + 64],
                                         precipb[:], start=True, stop=True)
                        sts = asb.tile([64, 512], BF16, tag="stS", bufs=3,
                                       name="sts")
                        nc.vector.tensor_mul(sts[:], pstages[s][:], rexp[0:64, :])
                        ag_dmas[pc // 2].append(nc.scalar.dma_start(
                            ag_in[pc // 2, 64 * s:64 * (s + 1),
                                  (pc % 2) * CHUNK:(pc % 2 + 1) * CHUNK], sts[:]))

                pending = None
                ag_dmas = [[], []]
                cc_insts = []
                for c in range(NCH):
                    for wk_i in range(4):
                        load_wo(4 * c + wk_i)
                    stages = []
                    denoms8 = asb.tile([8, 512], F32, tag="denoms", bufs=3,
                                       name="denoms8")
                    for sp in range(4):
                        aps = [apsum.tile([128, 512], F32, tag="attn", bufs=2,
                                          name=f"attn{j}")
                               for j in range(2)]
                        for kt in range(4 * c + 4):
                            vs = max(0, 128 * kt - CHUNK * c)
                            spt = apsum.tile([128, 1024], F32, tag="sps",
                                             bufs=3, name="spt")
                            for j in range(2):
                                nc.tensor.matmul(
                                    spt[:, 512 * j + vs:512 * j + 512],
                                    kt_sb[64 * j:64 * j + 64, kt * 128:(kt + 1) * 128],
                                    qt_sb[64 * j:64 * j + 64, sp,
                                          c * CHUNK + vs:(c + 1) * CHUNK],
                                    start=True, stop=True)
                            pt = asb.tile([128, 1024], BF16, tag="pT", bufs=8,
                                          name="pt")
                            nc.scalar.activation(
                                pt[:].rearrange("p (h q) -> p h q", h=2)[:, :, vs:512],
                                spt[:].rearrange("p (h q) -> p h q", h=2)[:, :, vs:512],
                                Exp, scale=0.125)
                            for j in range(2):
                                if kt >= 4 * c:
                                    nc.gpsimd.affine_select(
                                        out=pt[:, 512 * j + vs:512 * j + vs + 128],
                                        in_=pt[:, 512 * j + vs:512 * j + vs + 128],
                                        compare_op=mybir.AluOpType.is_ge,
                                        fill=0.0, base=0,
                                        pattern=[[1, 128]], channel_multiplier=-1)
                                nc.tensor.matmul(
                                    aps[j][0:65, vs:512],
                                    v_sb[:, kt, 65 * j:65 * j + 65],
                                    pt[:, 512 * j + vs:512 * j + 512],
                                    start=(kt == 0), stop=(kt == 4 * c + 3))
                        for j in range(2):
                            s = 2 * sp + j
                            stg = asb.tile([64, 512], BF16, tag="stage", bufs=12,
                                           name="stg")
                            nc.vector.tensor_copy(stg[:], aps[j][0:64, :])
                            dstg = asb.tile([128, 512], F32, tag="dstage", bufs=3,
                                            name="dstg")
                            nc.vector.tensor_copy(dstg[64:65, :], aps[j][64:65, :])
                            nc.scalar.dma_start(denoms8[s:s + 1, :], dstg[64:65, :])
                            stages.append(stg)
                        if PIPELINE_SCALE and sp == 0 and pending is not None:
                            pc_done = pending[0]
                            emit_scale(*pending)
                            pending = None
                            if pc_done == 1:
                                emit_ag(0)
                    recip8 = asb.tile([8, 512], F32, tag="recip", bufs=2,
                                      name="recip8")
                    nc.vector.reciprocal(recip8[:], denoms8[:])
                    recip8b = asb.tile([8, 512], BF16, tag="recipb", bufs=2,
                                       name="recip8b")
                    nc.gpsimd.tensor_copy(recip8b[:], recip8[:])
                    pending = (c, stages, recip8b)
                    if not PIPELINE_SCALE:
                        emit_scale(*pending)
                        pending = None
                if pending is not None:
                    emit_scale(*pending)
                emit_ag(1)

            # ---- output projection for my 512 rows ----
            gr_reg = nc.scalar.alloc_register("gr_reg")
            nc.scalar.reg_load(gr_reg, goffr_d[0:1, 0:1])
            goffr = nc.scalar.snap(gr_reg, donate=True, min_val=0, max_val=S)
            gc_reg = nc.scalar.alloc_register("gc_reg")
            nc.scalar.reg_load(gc_reg, goffc_d[0:1, 0:1])
            goffc = nc.scalar.snap(gc_reg, donate=True, min_val=0, max_val=CHUNK)
            gflat = gath[:].rearrange("h r q -> (h r) q")
            with tc.tile_pool(name="osb", bufs=1) as osb, \
                 tc.tile_pool(name="opsum", bufs=1, space="PSUM") as opsum:
                ag_sb = osb.tile([128, DT, CHUNK], BF16)
                for kt in range(DT):
                    d = nc.scalar.dma_start(
                        ag_sb[:, kt, :],
                        gflat[bass.ds(goffr + 128 * kt, 128),
                              bass.ds(goffc, CHUNK)])
                    for cc in cc_insts:
                        add_dep_helper(d.ins, cc.ins, sync=True,
                                       reason="gather read waits all AGs")
                for mp in range(2):
                    ops = [opsum.tile([128, 512], F32, tag="wops", bufs=8,
                                      name=f"wops{i}")
                           for i in range(8)]
                    for kt in range(DT):
                        for mi in range(2):
                            mt = 2 * mp + mi
                            st = ag_sb[:, kt, mt * 128:(mt + 1) * 128]
                            for nb in range(4):
                                nc.tensor.matmul(
                                    ops[4 * mi + nb][:],
                                    st, wo_sb[:, kt, nb * 512:(nb + 1) * 512],
                                    start=(kt == 0), stop=(kt == DT - 1))
                    for mi in range(2):
                        mt = 2 * mp + mi
                        outs = osb.tile([128, DIM], F32, tag="outs", bufs=2,
                                        name="outs")
                        for nb in range(4):
                            nc.scalar.activation(outs[:, nb * 512:(nb + 1) * 512],
                                                 ops[4 * mi + nb][:], Copy)
                        nc.scalar.dma_start(out_d[mt * 128:(mt + 1) * 128, :], outs[:])

    nc.finalize()
    return nc


_NC_CACHE = None


def _get_nc():
    global _NC_CACHE
    if _NC_CACHE is None:
        _NC_CACHE = _build()
    return _NC_CACHE


def _shard_inputs(x, wq, wk, wv, wo, freqs_cos, freqs_sin):
    """Pure layout work: slice batch, pick each core's heads, permute rope
    pairs within each head, permute wo rows to match the slot order."""
    x = np.ascontiguousarray(np.asarray(x, dtype=np.float32))
    wq = np.asarray(wq, dtype=np.float32)
    wk = np.asarray(wk, dtype=np.float32)
    wv = np.asarray(wv, dtype=np.float32)
    wo = np.asarray(wo, dtype=np.float32)
    cos = np.ascontiguousarray(np.asarray(freqs_cos, dtype=np.float32))
    sin = np.ascontiguousarray(np.asarray(freqs_sin, dtype=np.float32))

    # wo rows permuted once: gathered row 512*g + 64*s + d  <-  head 8g+slot(s)
    wo_perm = np.empty_like(wo)
    for g in range(4):
        for s_ in range(8):
            h = 8 * g + SLOT_TO_LOCAL[s_]
            wo_perm[512 * g + 64 * s_: 512 * g + 64 * (s_ + 1), :] = \
                wo[64 * h: 64 * (h + 1), :]
    wo_perm = np.ascontiguousarray(wo_perm)

    in_maps = []
    for core in range(N_CORES):
        b, g = core // 4, core % 4
        wq_cols = []
        for s_ in range(8):
            h = 8 * g + SLOT_TO_LOCAL[s_]
            wq_cols.append(wq[:, 64 * h + HD_PERM])
        wq_s = np.ascontiguousarray(np.concatenate(wq_cols, axis=1))
        wk_cols = [wk[:, 64 * (2 * g + j) + HD_PERM] for j in range(2)]
        wv_cols = wv[:, 64 * 2 * g: 64 * (2 * g + 2)]
        wkv_s = np.ascontiguousarray(
            np.concatenate(wk_cols + [wv_cols], axis=1))
        in_maps.append({
            "x": x[b], "wq": wq_s, "wkv": wkv_s, "wo": wo_perm,
            "cos": cos, "sin": sin,
            "goffr": np.array([[S * (g // 2)]], dtype=np.uint32),
            "goffc": np.array([[CHUNK * (g % 2)]], dtype=np.uint32),
        })
    return in_maps


def kernel(x, wq, wk, wv, wo, freqs_cos, freqs_sin, mask=None, start_pos=0,
           **_unused):
    nc = _get_nc()
    in_maps = _shard_inputs(x, wq, wk, wv, wo, freqs_cos, freqs_sin)
    res = bass_utils.run_bass_kernel_spmd(
        nc, in_maps, core_ids=list(range(N_CORES)))
    out = np.empty((B, S, DIM), dtype=np.float32)
    for core in range(N_CORES):
        b, g = core // 4, core % 4
        out[b, CHUNK * g: CHUNK * (g + 1), :] = res.results[core]["out"]
    return out

